# revision 10
# baseline (speedup 1.0000x reference)
"""Trainium2 Bass kernel for nn_GTAM_21852793602070 (dense_transformer).

GTAM block = CTA (channel-transposed attention) * 0.01 + PTA (patch attention).
With H=W=80 < PATCH=160, PTA is one full 6400-token attention per batch image.

Sharding (8 cores): core i handles batch b=i//4 and PTA-query slice
qi=i%4 (1600 positions). Conv weights replicated; each core computes the
full k/v (PTA) and q/k (CTA) convs for its batch, plus q/v on its slice.

Device decomposition per core (all matmuls on PE in float32r):
 - conv1x1 + depthwise3x3 fused into a dense 3x3 conv (9 tap-accumulated
   matmuls, contraction over 97 channels: 96 input + 1 validity channel
   that carries the conv1x1 bias through zero-padding exactly).
 - PTA: S^T chunks [128 keys, 400 queries] = k_chunk^T q on PE, exp on
   ScalarE (no max-subtraction: |S| < 0.011), PV accumulation with
   proj_w folded into v' and a ones-column producing the softmax
   denominator for free. Final transpose to position-major + normalize.
 - CTA: dots[96,96] accumulated from PE-transposed bf16 q/k chunks,
   softmax via Exp+accum_out, attn@v, proj emitted position-major.
"""

import os
import numpy as np

C = 96
B, H, W = 2, 80, 80
HW = H * W            # 6400
QS = HW // 4          # 1600 queries per core
NCORES = 8
QROWS = QS // W       # 20 image rows per core slice

_cache = {}
last_results = None   # BassKernelResults from the most recent run (for test.py)


def _host_prep(inputs):
    """Build the derived host-side tensors (weight fusion, padding, slicing)."""
    x = np.ascontiguousarray(np.asarray(inputs['x'], dtype=np.float32))
    XA = np.zeros((B, C + 1, 82, 82), np.float32)
    XA[:, :C, 1:81, 1:81] = x
    XA[:, C, 1:81, 1:81] = 1.0

    def fuse(qkv_w, qkv_b, dw_w):
        w1 = np.asarray(qkv_w, np.float32)[:, :, 0, 0]      # [288, 96]
        dw = np.asarray(dw_w, np.float32)[:, 0]             # [288, 3, 3]
        qb = np.asarray(qkv_b, np.float32)
        Wf = np.zeros((C + 1, 9, 3 * C), np.float32)
        for t in range(9):
            ty, tx = divmod(t, 3)
            Wf[:C, t, :] = (w1 * dw[:, ty, tx][:, None]).T
            Wf[C, t, :] = qb * dw[:, ty, tx]
        return Wf

    import ml_dtypes
    prep = {
        'wpta': fuse(inputs['pta_qkv_w'], inputs['pta_qkv_b'], inputs['pta_dw_w']),
        'wcta': fuse(inputs['cta_qkv_w'], inputs['cta_qkv_b'], inputs['cta_dw_w']),
        # [96, 3]: col g = dw_b[g*96:(g+1)*96]
        'bpta': np.ascontiguousarray(
            np.asarray(inputs['pta_dw_b'], np.float32).reshape(3, C).T),
        'bcta': np.ascontiguousarray(
            np.asarray(inputs['cta_dw_b'], np.float32).reshape(3, C).T),
        'wvproj': np.ascontiguousarray(np.concatenate(
            [np.asarray(inputs['pta_proj_w'], np.float32)[:, :, 0, 0].T,
             np.zeros((C, 2), np.float32)], axis=1)),  # [96, 98]: even N for fp32r
        'wctaproj': np.ascontiguousarray(
            np.asarray(inputs['cta_proj_w'], np.float32)[:, :, 0, 0].T),  # [96, 96]
        'bcomb': (np.asarray(inputs['pta_proj_b'], np.float32)
                  + 0.01 * np.asarray(inputs['cta_proj_b'], np.float32)),  # [96]
        'identr': np.eye(128, dtype=np.float32),
        'identb': np.eye(128, dtype=ml_dtypes.bfloat16),
        'XA': XA,
    }
    return prep


def _build_bass():
    import concourse.bass as bass
    from concourse import bacc
    import concourse.mybir as mybir
    import concourse.tile as tile
    from contextlib import ExitStack

    f32 = mybir.dt.float32
    f32r = mybir.dt.float32r
    bf16 = mybir.dt.bfloat16
    AF = mybir.ActivationFunctionType
    OP = mybir.AluOpType

    nc = bacc.Bacc("TRN2", target_bir_lowering=False)

    # ---- DRAM I/O ----
    d_xa = nc.dram_tensor("xa", [C + 1, 82, 82], f32r, kind="ExternalInput")
    d_xq = nc.dram_tensor("xq", [C + 1, QROWS + 2, 82], f32r, kind="ExternalInput")
    d_wpta = nc.dram_tensor("wpta", [C + 1, 9, 3 * C], f32r, kind="ExternalInput")
    d_wcta = nc.dram_tensor("wcta", [C + 1, 9, 3 * C], f32r, kind="ExternalInput")
    d_bpta = nc.dram_tensor("bpta", [C, 3], f32, kind="ExternalInput")
    d_bcta = nc.dram_tensor("bcta", [C, 3], f32, kind="ExternalInput")
    d_wvproj = nc.dram_tensor("wvproj", [C, C + 2], f32r, kind="ExternalInput")
    d_wctaproj = nc.dram_tensor("wctaproj", [C, C], f32r, kind="ExternalInput")
    d_bcomb = nc.dram_tensor("bcomb", [C], f32, kind="ExternalInput")
    d_identr = nc.dram_tensor("identr", [128, 128], f32, kind="ExternalInput")
    d_identb = nc.dram_tensor("identb", [128, 128], bf16, kind="ExternalInput")
    d_out = nc.dram_tensor("out", [QS, C], f32, kind="ExternalOutput")

    # full-image conv row chunks (6 rows = 480 cols per matmul) and slice chunks
    FULL_RC = [(r, 6) for r in range(0, 78, 6)] + [(78, 2)]
    SLICE_RC = [(0, 6), (6, 6), (12, 6), (18, 2)]
    # query free-dim chunks for PTA attention
    NQC = 4
    QCW = QS // NQC      # 400
    # position chunks for the final transpose/combine
    POSC = [(i * 128, 128) for i in range(12)] + [(1536, 64)]

    with tile.TileContext(nc) as tc, ExitStack() as top:
        consts = top.enter_context(tc.tile_pool(name="consts", bufs=1))
        big = top.enter_context(tc.tile_pool(name="big", bufs=1))

        # ---- load constants ----
        # All const loads go through the single SWDGE queue in this order, so
        # the first conv matmul's wait (on xa/wpta, queued last) transitively
        # covers every earlier const: fp32r self-loading matmuls only support
        # ONE sync wait, so no matmul may ever need a second DMA wait.
        bcomb_sb = consts.tile([128, C], f32)
        nc.gpsimd.dma_start(out=bcomb_sb, in_=d_bcomb.ap().partition_broadcast(128))
        identr_sb = consts.tile([128, 128], f32)
        nc.gpsimd.dma_start(identr_sb, d_identr.ap())
        identb_sb = consts.tile([128, 128], bf16)
        nc.gpsimd.dma_start(identb_sb, d_identb.ap())
        wctaproj_sb = consts.tile([C, C], f32r)
        nc.gpsimd.dma_start(wctaproj_sb, d_wctaproj.ap())
        wvproj_sb = consts.tile([C, C + 2], f32r)
        nc.gpsimd.dma_start(wvproj_sb, d_wvproj.ap())
        bpta_sb = consts.tile([C, 3], f32)
        nc.gpsimd.dma_start(bpta_sb, d_bpta.ap())
        bcta_sb = consts.tile([C, 3], f32)
        nc.gpsimd.dma_start(bcta_sb, d_bcta.ap())
        xq_sb = consts.tile([C + 1, QROWS + 2, 82], f32r)
        nc.gpsimd.dma_start(xq_sb, d_xq.ap())
        wcta_sb = consts.tile([C + 1, 9, 3 * C], f32r)
        nc.gpsimd.dma_start(wcta_sb, d_wcta.ap())
        wpta_sb = consts.tile([C + 1, 9, 3 * C], f32r)
        nc.gpsimd.dma_start(wpta_sb, d_wpta.ap())
        xa_sb = consts.tile([C + 1, 82, 82], f32r)
        nc.gpsimd.dma_start(xa_sb, d_xa.ap())

        # ---- persistent working tensors ----
        k_sb = big.tile([C, HW], f32r)        # PTA k  (channel-major)
        v_sb = big.tile([C, HW], f32r)        # PTA v
        q_sb = big.tile([C, QS], f32r)        # PTA q slice
        cq_sb = big.tile([C, HW], bf16)      # CTA q (bf16: errors damped by 0.01)
        ck_sb = big.tile([C, HW], bf16)      # CTA k
        cv_sb = big.tile([C, QS], f32r)       # CTA v slice
        vp_sb = big.tile([128, 50, C + 2], f32r)   # PTA v' = v^T proj^T | 1
        av_sb = big.tile([C, QS], f32r)       # CTA attn@v
        ctaT_sb = big.tile([128, 13, C], f32)  # CTA out, position-major
        u_sb = big.tile([C + 1, QS], f32)    # PTA unnormalized out^T (+Z row)
        out_sb = big.tile([128, 13, C], f32)

        def conv_chain(src_sb, w_sb, b_sb, group, dest_sb, row_chunks, pool):
            """Fused 3x3 conv for output channel group g (96 wide)."""
            ch0 = group * C
            for (r0, nrows) in row_chunks:
                n = nrows * 80
                ps = pool.tile([128, 512], f32, tag="ps")
                for t in range(9):
                    ty, tx = divmod(t, 3)
                    nc.tensor.matmul(
                        ps[:C, :n],
                        w_sb[:, t, ch0:ch0 + C],
                        src_sb[:, ty + r0:ty + r0 + nrows, tx:tx + 80],
                        start=(t == 0), stop=(t == 8))
                nc.vector.tensor_scalar_add(
                    dest_sb[:, r0 * 80:r0 * 80 + n], ps[:C, :n],
                    b_sb[:, group:group + 1])

        # =========== phase A: convs + v' + full CTA ===========
        with ExitStack() as pA:
            psA = pA.enter_context(tc.tile_pool(name="psA", bufs=2, space="PSUM"))
            psDots = pA.enter_context(tc.tile_pool(name="psDots", bufs=1, space="PSUM"))
            tpool = pA.enter_context(tc.tile_pool(name="tpool", bufs=4))
            small = pA.enter_context(tc.tile_pool(name="small", bufs=1))

            # Observer dummies: fp32r self-loading matmuls allow only ONE
            # sync wait, so absorb each const's DMA-queue wait with a tiny
            # throwaway matmul before any real matmul needs it.
            dmy = psA.tile([128, 512], f32, tag="ps")
            for t_ in (xa_sb, xq_sb, wpta_sb, wcta_sb, wvproj_sb, wctaproj_sb):
                sl = t_[:2, 0, :2] if len(t_.shape) == 3 else t_[:2, :2]
                nc.tensor.matmul(dmy[:2, :2], sl, sl, start=True, stop=True)
            nc.tensor.matmul(dmy[:2, :2], identr_sb[:2, :2], identr_sb[:2, :2],
                             start=True, stop=True)
            nc.tensor.matmul(dmy[:2, :2], identb_sb[:2, :2], identb_sb[:2, :2],
                             start=True, stop=True)

            # PTA convs: k, v full
            conv_chain(xa_sb, wpta_sb, bpta_sb, 1, k_sb, FULL_RC, psA)
            conv_chain(xa_sb, wpta_sb, bpta_sb, 2, v_sb, FULL_RC, psA)

            # PTA v' = v_chunk^T @ [proj^T | 0]
            for kc in range(50):
                ps = psA.tile([128, 512], f32, tag="ps")
                nc.tensor.matmul(ps[:, :C + 2], v_sb[:, kc * 128:kc * 128 + 128],
                                 wvproj_sb, start=True, stop=True)
                nc.vector.tensor_copy(vp_sb[:, kc, 0:C + 2], ps[:, 0:C + 2])
            # overwrite the junk 97th column with the softmax-denominator ones
            # (memset can't write f32r: memset f32 then converting copy)
            ones_sb = small.tile([128, 50, 1], f32)
            nc.vector.memset(ones_sb, 1.0)
            nc.vector.tensor_copy(vp_sb[:, :, C:C + 1], ones_sb)

            # PTA q on slice (emitted after v' so the S-matmul DVE wait
            # covers the vp evacuations)
            conv_chain(xq_sb, wpta_sb, bpta_sb, 0, q_sb, SLICE_RC, psA)

            # CTA convs: q, k full (bf16 dest); v on slice
            conv_chain(xa_sb, wcta_sb, bcta_sb, 0, cq_sb, FULL_RC, psA)
            conv_chain(xa_sb, wcta_sb, bcta_sb, 1, ck_sb, FULL_RC, psA)
            conv_chain(xq_sb, wcta_sb, bcta_sb, 2, cv_sb, SLICE_RC, psA)

            # CTA dots[96,96] accumulated over 50 position chunks
            dots_ps = psDots.tile([C, C], f32)
            for pc in range(50):
                sl = slice(pc * 128, pc * 128 + 128)
                tq = psA.tile([128, C], bf16, tag="tps")
                nc.tensor.transpose(tq, cq_sb[:, sl], identb_sb[:C, :C])
                qT = tpool.tile([128, C], bf16, tag="qT")
                nc.vector.tensor_copy(qT, tq)
                tk = psA.tile([128, C], bf16, tag="tps")
                nc.tensor.transpose(tk, ck_sb[:, sl], identb_sb[:C, :C])
                kT = tpool.tile([128, C], bf16, tag="kT")
                nc.vector.tensor_copy(kT, tk)
                nc.tensor.matmul(dots_ps, qT, kT,
                                 start=(pc == 0), stop=(pc == 49))

            # CTA softmax (free-dim) + attn^T
            attn_sb = small.tile([C, C], f32)
            z96 = small.tile([C, 1], f32)
            nc.scalar.activation(attn_sb, dots_ps, AF.Exp, accum_out=z96)
            zr96 = small.tile([C, 1], f32)
            nc.vector.reciprocal(zr96, z96)
            nc.vector.tensor_scalar_mul(attn_sb, attn_sb, zr96)
            tat = psA.tile([128, 512], f32, tag="ps")
            nc.tensor.transpose(tat[:C, :C], attn_sb, identr_sb[:C, :C])
            attnT_sb = small.tile([C, C], f32r)
            nc.vector.tensor_copy(attnT_sb, tat[:C, :C])

            # CTA attn@v on slice -> av_sb [96, 1600]
            for (o, n) in [(0, 512), (512, 512), (1024, 512), (1536, 64)]:
                ps = psA.tile([128, 512], f32, tag="ps")
                nc.tensor.matmul(ps[:C, :n], attnT_sb, cv_sb[:, o:o + n],
                                 start=True, stop=True)
                nc.vector.tensor_copy(av_sb[:, o:o + n], ps[:C, :n])

            # CTA proj, position-major: ctaT[n, j] = sum_c av[c, n] projT[c, j]
            for ci, (o, m) in enumerate(POSC):
                ps = psA.tile([128, 512], f32, tag="ps")
                nc.tensor.matmul(ps[:m, :C], av_sb[:, o:o + m],
                                 wctaproj_sb, start=True, stop=True)
                nc.vector.tensor_copy(ctaT_sb[:m, ci, :], ps[:m, :C])

        # =========== phase B: PTA attention ===========
        with ExitStack() as pB:
            psS = pB.enter_context(tc.tile_pool(name="psS", bufs=2, space="PSUM"))
            psU = pB.enter_context(tc.tile_pool(name="psU", bufs=1, space="PSUM"))
            ppool = pB.enter_context(tc.tile_pool(name="ppool", bufs=3))

            u_ps = psU.tile([C + 2, NQC, 512], f32)     # 4 banks, persists
            for _ in range(2):
                w = psS.tile([128, 2, 512], f32, tag="S")
                nc.vector.memset(w[:, :, :], 0.0)
            for qc in range(NQC):
                nc.scalar.copy(u_ps[:C + 1, qc, :QCW],
                               xa_sb[:, 5 * qc:5 * qc + 5, 0:80])
            for kc in range(50):
                ksl = slice(kc * 128, kc * 128 + 128)
                for h in range(2):
                    sps = psS.tile([128, 2, 512], f32, tag="S")
                    for i in range(2):
                        qc = h * 2 + i
                        nc.tensor.matmul(
                            sps[:, i, :QCW], k_sb[:, ksl],
                            q_sb[:, qc * QCW:(qc + 1) * QCW],
                            start=True, stop=True)
                    pt = ppool.tile([128, 2, QCW], f32r, tag="P")
                    nc.scalar.activation(pt, sps[:, :, :QCW], AF.Exp)
                    for i in range(2):
                        qc = h * 2 + i
                        nc.tensor.matmul(
                            u_ps[:, qc, :QCW], vp_sb[:, kc, :],
                            pt[:, i, :],
                            start=(kc == 0), stop=(kc == 49))
            for qc in range(NQC):
                nc.vector.tensor_copy(u_sb[:, qc * QCW:(qc + 1) * QCW],
                                      u_ps[:C + 1, qc, :QCW])

        # =========== phase C: transpose, normalize, combine, store ===========
        with ExitStack() as pC:
            psC = pC.enter_context(tc.tile_pool(name="psC", bufs=2, space="PSUM"))
            cpool = pC.enter_context(tc.tile_pool(name="cpool", bufs=3))

            for _ in range(2):
                w = psC.tile([128, C + 1], f32, tag="ptT")
                nc.vector.memset(w[:, :], 0.0)
            for ci, (o, m) in enumerate(POSC):
                ptT = psC.tile([128, C + 1], f32, tag="ptT")
                nc.tensor.transpose(ptT[:m, :], u_sb[:, o:o + m],
                                    identr_sb[:C + 1, :C + 1])
                ptf = cpool.tile([128, C + 1], f32, tag="ptf")
                nc.vector.tensor_copy(ptf[:m, :], ptT[:m, :])
                zr = cpool.tile([128, 1], f32, tag="zr")
                nc.vector.reciprocal(zr[:m], ptf[:m, C:C + 1])
                t1 = cpool.tile([128, C], f32, tag="t1")
                nc.vector.tensor_scalar_mul(t1[:m, :], ptf[:m, 0:C], zr[:m])
                t2 = cpool.tile([128, C], f32, tag="t2")
                nc.vector.scalar_tensor_tensor(
                    t2[:m, :], ctaT_sb[:m, ci, :], 0.01, t1[:m, :],
                    op0=OP.mult, op1=OP.add)
                nc.vector.tensor_add(out_sb[:m, ci, :], t2[:m, :],
                                     bcomb_sb[:m, :])

            nc.sync.dma_start(
                d_out.ap()[0:1536].rearrange("(n p) c -> p n c", p=128),
                out_sb[:, 0:12, :])
            nc.sync.dma_start(d_out.ap()[1536:1600], out_sb[0:64, 12, :])

    nc.compile()
    return nc


def _get_nc():
    if 'nc' not in _cache:
        _cache['nc'] = _build_bass()
    return _cache['nc']


def kernel(**inputs) -> np.ndarray:
    global last_results
    from concourse.bass_utils import run_bass_kernel_spmd

    prep = _host_prep(inputs)
    nc = _get_nc()

    in_maps = []
    for core in range(NCORES):
        b, qi = divmod(core, 4)
        in_maps.append({
            'xa': prep['XA'][b],
            'xq': np.ascontiguousarray(
                prep['XA'][b][:, qi * QROWS: qi * QROWS + QROWS + 2, :]),
            'wpta': prep['wpta'], 'wcta': prep['wcta'],
            'bpta': prep['bpta'], 'bcta': prep['bcta'],
            'wvproj': prep['wvproj'], 'wctaproj': prep['wctaproj'],
            'bcomb': prep['bcomb'],
            'identr': prep['identr'], 'identb': prep['identb'],
        })

    trace = bool(int(os.environ.get('GTAM_TRACE', '0')))
    res = run_bass_kernel_spmd(nc, in_maps, core_ids=list(range(NCORES)),
                               trace=trace)
    last_results = res

    out = np.zeros((B, HW, C), np.float32)
    for core in range(NCORES):
        b, qi = divmod(core, 4)
        out[b, qi * QS:(qi + 1) * QS] = res.results[core]['out']
    return out


# revision 11
# speedup vs baseline: 1.0074x; 1.0074x over previous
"""Trainium2 Bass kernel for nn_GTAM_21852793602070 (dense_transformer).

GTAM block = CTA (channel-transposed attention) * 0.01 + PTA (patch attention).
With H=W=80 < PATCH=160, PTA is one full 6400-token attention per batch image.

Sharding (8 cores): core i handles batch b=i//4 and PTA-query slice
qi=i%4 (1600 positions). Conv weights replicated; each core computes the
full k/v (PTA) and q/k (CTA) convs for its batch, plus q/v on its slice.

Device decomposition per core (all matmuls on PE in float32r):
 - conv1x1 + depthwise3x3 fused into a dense 3x3 conv (9 tap-accumulated
   matmuls, contraction over 97 channels: 96 input + 1 validity channel
   that carries the conv1x1 bias through zero-padding exactly).
 - PTA: S^T chunks [128 keys, 400 queries] = k_chunk^T q on PE, exp on
   ScalarE (no max-subtraction: |S| < 0.011), PV accumulation with
   proj_w folded into v' and a ones-column producing the softmax
   denominator for free. Final transpose to position-major + normalize.
 - CTA: dots[96,96] accumulated from PE-transposed bf16 q/k chunks,
   softmax via Exp+accum_out, attn@v, proj emitted position-major.
"""

import os
import numpy as np

C = 96
B, H, W = 2, 80, 80
HW = H * W            # 6400
QS = HW // 4          # 1600 queries per core
NCORES = 8
QROWS = QS // W       # 20 image rows per core slice

_cache = {}
last_results = None   # BassKernelResults from the most recent run (for test.py)


def _host_prep(inputs):
    """Build the derived host-side tensors (weight fusion, padding, slicing)."""
    x = np.ascontiguousarray(np.asarray(inputs['x'], dtype=np.float32))
    XA = np.zeros((B, C + 1, 82, 82), np.float32)
    XA[:, :C, 1:81, 1:81] = x
    XA[:, C, 1:81, 1:81] = 1.0

    def fuse(qkv_w, qkv_b, dw_w):
        w1 = np.asarray(qkv_w, np.float32)[:, :, 0, 0]      # [288, 96]
        dw = np.asarray(dw_w, np.float32)[:, 0]             # [288, 3, 3]
        qb = np.asarray(qkv_b, np.float32)
        Wf = np.zeros((C + 1, 9, 3 * C), np.float32)
        for t in range(9):
            ty, tx = divmod(t, 3)
            Wf[:C, t, :] = (w1 * dw[:, ty, tx][:, None]).T
            Wf[C, t, :] = qb * dw[:, ty, tx]
        return Wf

    import ml_dtypes
    prep = {
        'wpta': fuse(inputs['pta_qkv_w'], inputs['pta_qkv_b'], inputs['pta_dw_w']),
        'wcta': fuse(inputs['cta_qkv_w'], inputs['cta_qkv_b'], inputs['cta_dw_w']),
        # [96, 3]: col g = dw_b[g*96:(g+1)*96]
        'bpta': np.ascontiguousarray(
            np.asarray(inputs['pta_dw_b'], np.float32).reshape(3, C).T),
        'bcta': np.ascontiguousarray(
            np.asarray(inputs['cta_dw_b'], np.float32).reshape(3, C).T),
        'wvproj': np.ascontiguousarray(np.concatenate(
            [np.asarray(inputs['pta_proj_w'], np.float32)[:, :, 0, 0].T,
             np.zeros((C, 2), np.float32)], axis=1)),  # [96, 98]: even N for fp32r
        'wctaproj': np.ascontiguousarray(
            np.asarray(inputs['cta_proj_w'], np.float32)[:, :, 0, 0].T),  # [96, 96]
        'bcomb': (np.asarray(inputs['pta_proj_b'], np.float32)
                  + 0.01 * np.asarray(inputs['cta_proj_b'], np.float32)),  # [96]
        'identr': np.eye(128, dtype=np.float32),
        'XAb': XA.astype(ml_dtypes.bfloat16),
        'wctab': None,  # filled below
        'identb': np.eye(128, dtype=ml_dtypes.bfloat16),
        'XA': XA,
    }
    prep['wctab'] = prep['wcta'].astype(ml_dtypes.bfloat16)
    return prep


def _build_bass():
    import concourse.bass as bass
    from concourse import bacc
    import concourse.mybir as mybir
    import concourse.tile as tile
    from contextlib import ExitStack

    f32 = mybir.dt.float32
    f32r = mybir.dt.float32r
    bf16 = mybir.dt.bfloat16
    AF = mybir.ActivationFunctionType
    OP = mybir.AluOpType

    nc = bacc.Bacc("TRN2", target_bir_lowering=False)

    # ---- DRAM I/O ----
    d_xa = nc.dram_tensor("xa", [C + 1, 82, 82], f32r, kind="ExternalInput")
    d_xq = nc.dram_tensor("xq", [C + 1, QROWS + 2, 82], f32r, kind="ExternalInput")
    d_wpta = nc.dram_tensor("wpta", [C + 1, 9, 3 * C], f32r, kind="ExternalInput")
    d_wcta = nc.dram_tensor("wcta", [C + 1, 9, 3 * C], bf16, kind="ExternalInput")
    d_xab = nc.dram_tensor("xab", [C + 1, 82, 82], bf16, kind="ExternalInput")
    d_xqb = nc.dram_tensor("xqb", [C + 1, QROWS + 2, 82], bf16, kind="ExternalInput")
    d_bpta = nc.dram_tensor("bpta", [C, 3], f32, kind="ExternalInput")
    d_bcta = nc.dram_tensor("bcta", [C, 3], f32, kind="ExternalInput")
    d_wvproj = nc.dram_tensor("wvproj", [C, C + 2], f32r, kind="ExternalInput")
    d_wctaproj = nc.dram_tensor("wctaproj", [C, C], f32r, kind="ExternalInput")
    d_bcomb = nc.dram_tensor("bcomb", [C], f32, kind="ExternalInput")
    d_identr = nc.dram_tensor("identr", [128, 128], f32, kind="ExternalInput")
    d_identb = nc.dram_tensor("identb", [128, 128], bf16, kind="ExternalInput")
    d_out = nc.dram_tensor("out", [QS, C], f32, kind="ExternalOutput")

    # full-image conv row chunks (6 rows = 480 cols per matmul) and slice chunks
    FULL_RC = [(r, 6) for r in range(0, 78, 6)] + [(78, 2)]
    SLICE_RC = [(0, 6), (6, 6), (12, 6), (18, 2)]
    # query free-dim chunks for PTA attention
    NQC = 4
    QCW = QS // NQC      # 400
    # position chunks for the final transpose/combine
    POSC = [(i * 128, 128) for i in range(12)] + [(1536, 64)]

    with tile.TileContext(nc) as tc, ExitStack() as top:
        consts = top.enter_context(tc.tile_pool(name="consts", bufs=1))
        big = top.enter_context(tc.tile_pool(name="big", bufs=1))

        # ---- load constants ----
        # All const loads go through the single SWDGE queue in this order, so
        # the first conv matmul's wait (on xa/wpta, queued last) transitively
        # covers every earlier const: fp32r self-loading matmuls only support
        # ONE sync wait, so no matmul may ever need a second DMA wait.
        bcomb_sb = consts.tile([128, C], f32)
        nc.gpsimd.dma_start(out=bcomb_sb, in_=d_bcomb.ap().partition_broadcast(128))
        identr_sb = consts.tile([128, 128], f32)
        nc.gpsimd.dma_start(identr_sb, d_identr.ap())
        identb_sb = consts.tile([128, 128], bf16)
        nc.gpsimd.dma_start(identb_sb, d_identb.ap())
        wctaproj_sb = consts.tile([C, C], f32r)
        nc.gpsimd.dma_start(wctaproj_sb, d_wctaproj.ap())
        wvproj_sb = consts.tile([C, C + 2], f32r)
        nc.gpsimd.dma_start(wvproj_sb, d_wvproj.ap())
        bpta_sb = consts.tile([C, 3], f32)
        nc.gpsimd.dma_start(bpta_sb, d_bpta.ap())
        bcta_sb = consts.tile([C, 3], f32)
        nc.gpsimd.dma_start(bcta_sb, d_bcta.ap())
        xq_sb = consts.tile([C + 1, QROWS + 2, 82], f32r)
        nc.gpsimd.dma_start(xq_sb, d_xq.ap())
        wcta_sb = consts.tile([C + 1, 9, 3 * C], bf16)
        nc.gpsimd.dma_start(wcta_sb, d_wcta.ap())
        xab_sb = consts.tile([C + 1, 82, 82], bf16)
        nc.gpsimd.dma_start(xab_sb, d_xab.ap())
        xqb_sb = consts.tile([C + 1, QROWS + 2, 82], bf16)
        nc.gpsimd.dma_start(xqb_sb, d_xqb.ap())
        wpta_sb = consts.tile([C + 1, 9, 3 * C], f32r)
        nc.gpsimd.dma_start(wpta_sb, d_wpta.ap())
        xa_sb = consts.tile([C + 1, 82, 82], f32r)
        nc.gpsimd.dma_start(xa_sb, d_xa.ap())

        # ---- persistent working tensors ----
        k_sb = big.tile([C, HW], f32r)        # PTA k  (channel-major)
        v_sb = big.tile([C, HW], f32r)        # PTA v
        q_sb = big.tile([C, QS], f32r)        # PTA q slice
        cq_sb = big.tile([C, HW], bf16)      # CTA q (bf16: errors damped by 0.01)
        ck_sb = big.tile([C, HW], bf16)      # CTA k
        cv_sb = big.tile([C, QS], f32r)       # CTA v slice
        vp_sb = big.tile([128, 50, C + 2], f32r)   # PTA v' = v^T proj^T | 1
        av_sb = big.tile([C, QS], f32r)       # CTA attn@v
        ctaT_sb = big.tile([128, 13, C], f32)  # CTA out, position-major
        u_sb = big.tile([C + 1, QS], f32)    # PTA unnormalized out^T (+Z row)
        out_sb = big.tile([128, 13, C], f32)

        def conv_chain(src_sb, w_sb, b_sb, group, dest_sb, row_chunks, pool):
            """Fused 3x3 conv for output channel group g (96 wide)."""
            ch0 = group * C
            for (r0, nrows) in row_chunks:
                n = nrows * 80
                ps = pool.tile([128, 512], f32, tag="ps")
                for t in range(9):
                    ty, tx = divmod(t, 3)
                    nc.tensor.matmul(
                        ps[:C, :n],
                        w_sb[:, t, ch0:ch0 + C],
                        src_sb[:, ty + r0:ty + r0 + nrows, tx:tx + 80],
                        start=(t == 0), stop=(t == 8))
                nc.vector.tensor_scalar_add(
                    dest_sb[:, r0 * 80:r0 * 80 + n], ps[:C, :n],
                    b_sb[:, group:group + 1])

        # =========== phase A: convs + v' + full CTA ===========
        with ExitStack() as pA:
            psA = pA.enter_context(tc.tile_pool(name="psA", bufs=2, space="PSUM"))
            psDots = pA.enter_context(tc.tile_pool(name="psDots", bufs=1, space="PSUM"))
            tpool = pA.enter_context(tc.tile_pool(name="tpool", bufs=4))
            small = pA.enter_context(tc.tile_pool(name="small", bufs=1))

            # Observer dummies: fp32r self-loading matmuls allow only ONE
            # sync wait, so absorb each const's DMA-queue wait with a tiny
            # throwaway matmul before any real matmul needs it.
            dmy = psA.tile([128, 512], f32, tag="ps")
            for t_ in (xa_sb, xq_sb, wpta_sb, wcta_sb, xab_sb, xqb_sb,
                       wvproj_sb, wctaproj_sb):
                sl = t_[:2, 0, :2] if len(t_.shape) == 3 else t_[:2, :2]
                nc.tensor.matmul(dmy[:2, :2], sl, sl, start=True, stop=True)
            nc.tensor.matmul(dmy[:2, :2], identr_sb[:2, :2], identr_sb[:2, :2],
                             start=True, stop=True)
            nc.tensor.matmul(dmy[:2, :2], identb_sb[:2, :2], identb_sb[:2, :2],
                             start=True, stop=True)

            # PTA convs: k, v full
            conv_chain(xa_sb, wpta_sb, bpta_sb, 1, k_sb, FULL_RC, psA)
            conv_chain(xa_sb, wpta_sb, bpta_sb, 2, v_sb, FULL_RC, psA)

            # PTA v' = v_chunk^T @ [proj^T | 0]
            for kc in range(50):
                ps = psA.tile([128, 512], f32, tag="ps")
                nc.tensor.matmul(ps[:, :C + 2], v_sb[:, kc * 128:kc * 128 + 128],
                                 wvproj_sb, start=True, stop=True)
                nc.vector.tensor_copy(vp_sb[:, kc, 0:C + 2], ps[:, 0:C + 2])
            # overwrite the junk 97th column with the softmax-denominator ones
            # (memset can't write f32r: memset f32 then converting copy)
            ones_sb = small.tile([128, 50, 1], f32)
            nc.vector.memset(ones_sb, 1.0)
            nc.vector.tensor_copy(vp_sb[:, :, C:C + 1], ones_sb)

            # PTA q on slice (emitted after v' so the S-matmul DVE wait
            # covers the vp evacuations)
            conv_chain(xq_sb, wpta_sb, bpta_sb, 0, q_sb, SLICE_RC, psA)

            # CTA convs: q, k full (bf16 dest); v on slice
            conv_chain(xab_sb, wcta_sb, bcta_sb, 0, cq_sb, FULL_RC, psA)
            conv_chain(xab_sb, wcta_sb, bcta_sb, 1, ck_sb, FULL_RC, psA)
            conv_chain(xqb_sb, wcta_sb, bcta_sb, 2, cv_sb, SLICE_RC, psA)

            # CTA dots[96,96] accumulated over 50 position chunks
            dots_ps = psDots.tile([C, C], f32)
            for pc in range(50):
                sl = slice(pc * 128, pc * 128 + 128)
                tq = psA.tile([128, C], bf16, tag="tps")
                nc.tensor.transpose(tq, cq_sb[:, sl], identb_sb[:C, :C])
                qT = tpool.tile([128, C], bf16, tag="qT")
                nc.vector.tensor_copy(qT, tq)
                tk = psA.tile([128, C], bf16, tag="tps")
                nc.tensor.transpose(tk, ck_sb[:, sl], identb_sb[:C, :C])
                kT = tpool.tile([128, C], bf16, tag="kT")
                nc.vector.tensor_copy(kT, tk)
                nc.tensor.matmul(dots_ps, qT, kT,
                                 start=(pc == 0), stop=(pc == 49))

            # CTA softmax (free-dim) + attn^T
            attn_sb = small.tile([C, C], f32)
            z96 = small.tile([C, 1], f32)
            nc.scalar.activation(attn_sb, dots_ps, AF.Exp, accum_out=z96)
            zr96 = small.tile([C, 1], f32)
            nc.vector.reciprocal(zr96, z96)
            nc.vector.tensor_scalar_mul(attn_sb, attn_sb, zr96)
            tat = psA.tile([128, 512], f32, tag="ps")
            nc.tensor.transpose(tat[:C, :C], attn_sb, identr_sb[:C, :C])
            attnT_sb = small.tile([C, C], f32r)
            nc.vector.tensor_copy(attnT_sb, tat[:C, :C])

            # CTA attn@v on slice -> av_sb [96, 1600]
            for (o, n) in [(0, 512), (512, 512), (1024, 512), (1536, 64)]:
                ps = psA.tile([128, 512], f32, tag="ps")
                nc.tensor.matmul(ps[:C, :n], attnT_sb, cv_sb[:, o:o + n],
                                 start=True, stop=True)
                nc.vector.tensor_copy(av_sb[:, o:o + n], ps[:C, :n])

            # CTA proj, position-major: ctaT[n, j] = sum_c av[c, n] projT[c, j]
            for ci, (o, m) in enumerate(POSC):
                ps = psA.tile([128, 512], f32, tag="ps")
                nc.tensor.matmul(ps[:m, :C], av_sb[:, o:o + m],
                                 wctaproj_sb, start=True, stop=True)
                nc.vector.tensor_copy(ctaT_sb[:m, ci, :], ps[:m, :C])

        # =========== phase B: PTA attention ===========
        with ExitStack() as pB:
            psS = pB.enter_context(tc.tile_pool(name="psS", bufs=2, space="PSUM"))
            psU = pB.enter_context(tc.tile_pool(name="psU", bufs=1, space="PSUM"))
            ppool = pB.enter_context(tc.tile_pool(name="ppool", bufs=3))

            u_ps = psU.tile([C + 2, NQC, 512], f32)     # 4 banks, persists
            for _ in range(2):
                w = psS.tile([128, 2, 512], f32, tag="S")
                nc.vector.memset(w[:, :, :], 0.0)
            for qc in range(NQC):
                nc.scalar.copy(u_ps[:C + 1, qc, :QCW],
                               xa_sb[:, 5 * qc:5 * qc + 5, 0:80])
            for kc in range(50):
                ksl = slice(kc * 128, kc * 128 + 128)
                for h in range(2):
                    sps = psS.tile([128, 2, 512], f32, tag="S")
                    for i in range(2):
                        qc = h * 2 + i
                        nc.tensor.matmul(
                            sps[:, i, :QCW], k_sb[:, ksl],
                            q_sb[:, qc * QCW:(qc + 1) * QCW],
                            start=True, stop=True)
                    pt = ppool.tile([128, 2, QCW], f32r, tag="P")
                    nc.scalar.activation(pt, sps[:, :, :QCW], AF.Exp)
                    for i in range(2):
                        qc = h * 2 + i
                        nc.tensor.matmul(
                            u_ps[:, qc, :QCW], vp_sb[:, kc, :],
                            pt[:, i, :],
                            start=(kc == 0), stop=(kc == 49))
            for qc in range(NQC):
                nc.vector.tensor_copy(u_sb[:, qc * QCW:(qc + 1) * QCW],
                                      u_ps[:C + 1, qc, :QCW])

        # =========== phase C: transpose, normalize, combine, store ===========
        with ExitStack() as pC:
            psC = pC.enter_context(tc.tile_pool(name="psC", bufs=2, space="PSUM"))
            cpool = pC.enter_context(tc.tile_pool(name="cpool", bufs=3))

            for _ in range(2):
                w = psC.tile([128, C + 1], f32, tag="ptT")
                nc.vector.memset(w[:, :], 0.0)
            for ci, (o, m) in enumerate(POSC):
                ptT = psC.tile([128, C + 1], f32, tag="ptT")
                nc.tensor.transpose(ptT[:m, :], u_sb[:, o:o + m],
                                    identr_sb[:C + 1, :C + 1])
                ptf = cpool.tile([128, C + 1], f32, tag="ptf")
                nc.vector.tensor_copy(ptf[:m, :], ptT[:m, :])
                zr = cpool.tile([128, 1], f32, tag="zr")
                nc.vector.reciprocal(zr[:m], ptf[:m, C:C + 1])
                t1 = cpool.tile([128, C], f32, tag="t1")
                nc.vector.tensor_scalar_mul(t1[:m, :], ptf[:m, 0:C], zr[:m])
                t2 = cpool.tile([128, C], f32, tag="t2")
                nc.vector.scalar_tensor_tensor(
                    t2[:m, :], ctaT_sb[:m, ci, :], 0.01, t1[:m, :],
                    op0=OP.mult, op1=OP.add)
                nc.vector.tensor_add(out_sb[:m, ci, :], t2[:m, :],
                                     bcomb_sb[:m, :])

            nc.sync.dma_start(
                d_out.ap()[0:1536].rearrange("(n p) c -> p n c", p=128),
                out_sb[:, 0:12, :])
            nc.sync.dma_start(d_out.ap()[1536:1600], out_sb[0:64, 12, :])

    nc.compile()
    return nc


def _get_nc():
    if 'nc' not in _cache:
        _cache['nc'] = _build_bass()
    return _cache['nc']


def kernel(**inputs) -> np.ndarray:
    global last_results
    from concourse.bass_utils import run_bass_kernel_spmd

    prep = _host_prep(inputs)
    nc = _get_nc()

    in_maps = []
    for core in range(NCORES):
        b, qi = divmod(core, 4)
        in_maps.append({
            'xa': prep['XA'][b],
            'xq': np.ascontiguousarray(
                prep['XA'][b][:, qi * QROWS: qi * QROWS + QROWS + 2, :]),
            'wpta': prep['wpta'], 'wcta': prep['wctab'],
            'xab': prep['XAb'][b],
            'xqb': np.ascontiguousarray(
                prep['XAb'][b][:, qi * QROWS: qi * QROWS + QROWS + 2, :]),
            'bpta': prep['bpta'], 'bcta': prep['bcta'],
            'wvproj': prep['wvproj'], 'wctaproj': prep['wctaproj'],
            'bcomb': prep['bcomb'],
            'identr': prep['identr'], 'identb': prep['identb'],
        })

    trace = bool(int(os.environ.get('GTAM_TRACE', '0')))
    res = run_bass_kernel_spmd(nc, in_maps, core_ids=list(range(NCORES)),
                               trace=trace)
    last_results = res

    out = np.zeros((B, HW, C), np.float32)
    for core in range(NCORES):
        b, qi = divmod(core, 4)
        out[b, qi * QS:(qi + 1) * QS] = res.results[core]['out']
    return out
